# revision 22
# baseline (speedup 1.0000x reference)
"""BlockwiseDense Trainium2 kernel (8 NeuronCores, sharded over out_blocks).

Math (per reference):
    w = rram_quantize(relu(cores))          # snap to 256 log-spaced levels
    y[b,i,j,k] = sum_l w[i,j,k,l] * x[b,j,l]

Quantization is done analytically instead of searchsorted+select:
    levels[m] = A - B*r^m,  r = exp(-tau/255),  B = scale,  A = g_min + B
    t(c) = log_r((A-c)/B)   (continuous inverse; monotone in c)
    nearest-level index n = floor(t + 1 - delta)  with constant
    delta = log_r((1+r)/2)  (midpoints are uniformly offset in t-space)
per element:   t' = Ln(c*(-1/B) + A/B)              (ACT pass 1)
               n  = sat_u8(rne(t'*MULT + C0))       (DVE; u8 convert is
                                                     RNE + saturate = clip)
               e  = Exp(n * ln_r)   in fp32         (ACT pass 2)
               q  = f16(A - B*e)                    (GpSimd tensor_scalar)
then fp16 matmuls (q @ x^T chunks) accumulate in fp32 PSUM; results are
evicted as fp16 and gathered/upcast host-side.

Sharding: core c takes out_blocks i in {2c, 2c+1}.  Host pre-transposes
x -> x^T (fp16) and cores -> (i, jp, l, jj, k) fp32 so DMA rows are 2KB
contiguous and the contraction index l lands on partitions.
"""

import numpy as np

import concourse.bacc as bacc
import concourse.mybir as mybir
from concourse.tile import TileContext
from concourse.bass_utils import run_bass_kernel_spmd

# ---- problem constants (hardcoded per contract) ----
BATCH = 128
IN_BLOCKS = 16
OUT_BLOCKS = 16
NB = 256  # block size (num_rows == num_cols)
N_CORES = 8
I_PER_CORE = OUT_BLOCKS // N_CORES  # 2
JP = IN_BLOCKS // 2  # j-pairs

TAU, G_INF, G_MIN, L = 0.75, 2.0, 0.001, 256
B_SCALE = (G_INF - G_MIN) / (1.0 - float(np.exp(-TAU)))
A_OFF = G_MIN + B_SCALE
MULT = -(L - 1) / TAU
LN_R = -TAU / (L - 1)
_r = float(np.exp(LN_R))
DELTA = float(np.log((1 + _r) / 2) / LN_R)
C0 = 0.5 - DELTA  # rne(s) == floor(s+0.5): n = floor(t + 1 - delta)

F32 = mybir.dt.float32
F16 = mybir.dt.float16
U8 = mybir.dt.uint8

_CACHE = {}


class _ForceCombinedLnExpTables:
    """Make the act-table-load pass resolve both Ln and Exp to the one set
    that contains both (natural_log_exp_and_others), instead of ping-ponging
    between the Exp-only and Ln-only sets on every use.  Indices into
    act_info.json must be preserved, so we blank Ln/Exp out of the other
    sets rather than reordering."""

    def __enter__(self):
        self._orig = bacc.get_activation_tables
        Ln = mybir.ActivationFunctionType.Ln
        Exp = mybir.ActivationFunctionType.Exp

        def patched(arch):
            tabs = self._orig(arch)
            out = {}
            for name, fns in tabs.items():
                if name != "natural_log_exp_and_others" and (Ln in fns or Exp in fns):
                    fns = fns - {Ln, Exp}
                out[name] = fns
            return out

        bacc.get_activation_tables = patched
        return self

    def __exit__(self, *exc):
        bacc.get_activation_tables = self._orig


def _build():
    nc = bacc.Bacc(trn_type="TRN2")
    P = 128

    # host layouts (per core):
    #   xt: (JP, 128, 4, BATCH) f16   -- x^T, 2KB rows per partition
    #   wt: (I, JP, 256, 2, NB) f32   -- cores^T, (i, jp, l, jj, k), 2KB rows
    #   y:  (BATCH, I*IN_BLOCKS*NB) f16
    xt_d = nc.dram_tensor("xt", [JP, P, 4, BATCH], F16, kind="ExternalInput")
    wt_d = nc.dram_tensor(
        "wt", [I_PER_CORE, JP, NB, 2, NB], F32, kind="ExternalInput"
    )
    y_d = nc.dram_tensor(
        "y", [BATCH, I_PER_CORE * IN_BLOCKS * NB], F16, kind="ExternalOutput"
    )

    with TileContext(nc) as tc:
        with (
            tc.tile_pool(name="singles", bufs=1) as singles,
            tc.tile_pool(name="wraw", bufs=6) as wpool,
            tc.tile_pool(name="xtp", bufs=6) as xpool,
            tc.tile_pool(name="quant", bufs=3) as qpool,
            tc.tile_pool(name="yout", bufs=4) as ypool,
            tc.tile_pool(name="yps", bufs=4, space="PSUM") as yps,
        ):
            bias_ln = singles.tile([P, 1], F32)
            nc.vector.memset(bias_ln[:], A_OFF / B_SCALE)

            yv = y_d.rearrange("b (i j k) -> b i j k", i=I_PER_CORE, k=NB)
            for jp in range(JP):
                njj = 2
                jjs = [0, 1]
                jj0 = 0

                # ---- load W^T pair: DMAs per (i, h) for queue spread.
                # The first pairs gate the whole pipeline, so split them into
                # many small DMAs issued from several (idle) engines: this
                # parallelizes both descriptor-issue and per-queue execution.
                wraw = wpool.tile([P, I_PER_CORE, 2, njj, NB], F32, tag="wraw")
                for i in range(I_PER_CORE):
                    src = wt_d[i, jp].rearrange("(h p) jj k -> p h jj k", p=P)
                    for h in range(2):
                        nc.sync.dma_start(out=wraw[:, i, h], in_=src[:, h])
                # x^T chunks for this pair (f16)
                xt_sb = xpool.tile([P, 2 * njj, BATCH], F16, tag="xt")
                nc.sync.dma_start(out=xt_sb[:], in_=xt_d[jp])

                # ---- quantize: Ln -> u8(rne+sat) -> Exp -> q16 ----
                flat = "p a b c k -> p (a b c k)"
                FD = njj * I_PER_CORE * 2 * NB
                tp = qpool.tile([P, FD], F32, tag="tp")
                nc.scalar.activation(
                    tp[:],
                    wraw[:].rearrange(flat),
                    mybir.ActivationFunctionType.Ln,
                    bias=bias_ln[:, 0:1],
                    scale=-1.0 / B_SCALE,
                )
                nu = qpool.tile([P, FD], U8, tag="nu")
                nc.vector.tensor_scalar(
                    nu[:], tp[:], MULT, C0, mybir.AluOpType.mult, mybir.AluOpType.add
                )
                ef = qpool.tile([P, FD], F32, tag="ef")
                nc.scalar.activation(
                    ef[:],
                    nu[:],
                    mybir.ActivationFunctionType.Exp,
                    bias=0.0,
                    scale=LN_R,
                )
                q16 = qpool.tile([P, I_PER_CORE, 2, njj, NB], F16, tag="q16")
                # last pair: DVE is ~2x faster than GpSimd here and sits on
                # the critical drain path at the kernel tail
                q16_eng = nc.vector if jp == JP - 1 else nc.gpsimd
                q16_eng.tensor_scalar(
                    q16[:].rearrange(flat),
                    ef[:],
                    -B_SCALE,
                    A_OFF,
                    mybir.AluOpType.mult,
                    mybir.AluOpType.add,
                )

                # ---- fp16 matmuls ----
                yp = yps.tile([P, I_PER_CORE * njj * NB], F32, tag="yp")
                for i in range(I_PER_CORE):
                    for jx, jj in enumerate(jjs):
                        for h in range(2):
                            s = (i * njj + jx) * NB
                            nc.tensor.matmul(
                                yp[:, s : s + NB],
                                xt_sb[:, 2 * jx + h, :],
                                q16[:, i, h, jx, :],
                                start=(h == 0),
                                stop=(h == 1),
                            )

                # ---- evict fp32 PSUM -> fp16 SBUF, store granule ----
                ysb = ypool.tile([P, I_PER_CORE, njj, NB], F16, tag="ysb")
                nc.vector.tensor_copy(
                    ysb[:].rearrange("p i jj k -> p (i jj k)"), yp[:]
                )
                nc.gpsimd.dma_start(
                    out=yv[:, :, 2 * jp + jj0 : 2 * jp + jj0 + njj, :],
                    in_=ysb[:],
                )

    with _ForceCombinedLnExpTables():
        nc.compile()
    return nc


def _get_nc():
    if "nc" not in _CACHE:
        _CACHE["nc"] = _build()
    return _CACHE["nc"]


def kernel(x: np.ndarray, cores: np.ndarray, _trace=False, _trace_kwargs=None):
    x = np.asarray(x, dtype=np.float32)
    cores = np.asarray(cores, dtype=np.float32)

    # x^T in fp16, laid out (jp, p, c, b) so each partition row is 2KB
    xt = np.ascontiguousarray(
        x.T.reshape(JP, 4, 128, BATCH).transpose(0, 2, 1, 3).astype(np.float16)
    )
    # cores^T: (i, j, l, k) then j-pair interleave -> (i, jp, l, jj, k)
    wt = np.ascontiguousarray(
        cores.transpose(0, 1, 3, 2)
        .reshape(OUT_BLOCKS, JP, 2, NB, NB)
        .transpose(0, 1, 3, 2, 4)
    )
    in_maps = [
        {"xt": xt, "wt": wt[c * I_PER_CORE : (c + 1) * I_PER_CORE]}
        for c in range(N_CORES)
    ]

    nc = _get_nc()
    kw = {}
    if _trace:
        kw = dict(trace=True, **(_trace_kwargs or {}))
    out = run_bass_kernel_spmd(nc, in_maps, core_ids=list(range(N_CORES)), **kw)
    if _trace:
        _CACHE["last_result"] = out
    y = np.concatenate(
        [
            r["y"].astype(np.float32).reshape(BATCH, I_PER_CORE, IN_BLOCKS, NB)
            for r in out.results
        ],
        axis=1,
    )
    return y
